# revision 13
# baseline (speedup 1.0000x reference)
"""Difference-of-Gaussians + 3x3x3 NMS mask on 8 Trainium2 NeuronCores.

Strategy (channel-parallel, per the sharding hint):
  - 50 DoG channels sharded across 8 cores (7 output channels each, with
    1-channel halos for the scale-direction maxpool). All cores run the
    IDENTICAL Bass program (SPMD); only input data differs.
  - The separable gaussian blur is done as two banded-Toeplitz matmul passes
    on the TensorEngine (fp32). Pass 2 folds the DoG subtraction and the
    sigma scaling into the PSUM accumulation (dog = A_d*(+s*G_d) + A_{d+1}*
    (-s*G_{d+1})), so the vector engines never see the subtraction.
  - The 3x3x3 maxpool decomposes into H-max (partition direction, shifts done
    with SBUF->SBUF DMAs), scale-max (channel-contiguous free-dim tiles), and
    W-max (free-dim shifted APs); mask = (dog == pooled) & (dog > thr).
"""

import numpy as np

import concourse.bass as bass
import concourse.mybir as mybir
from concourse import tile
from concourse.bass_utils import run_bass_kernel_spmd

F32 = mybir.dt.float32
U8 = mybir.dt.uint8
AX = mybir.AluOpType

H = W = 1024
NSTRIP = 8
SIGMA_BINS = 50
NSIG = 51           # gaussian channels
NDOG = 50           # dog channels
KRAD = 47           # max kernel radius: (taps-1)//2, taps=95
KTAPS = 95
THRESHOLD = 0.001
NEG = -3.0e38       # -inf stand-in (fp32 finite)
BIGBIAS = 1.0e30

NG = 10             # gaussian slots per core
ND = 9              # dog slots per core (7 out + 2 scale halo)
NOUT = 7            # output dog channels per core
STARTS = [0, 7, 14, 21, 28, 35, 42, 43]

# batches over dog slots: (dog_lo, n_slots, n_masks). masks are local slots
# [1, 1+nmask); gaussian slots used are [dog_lo, dog_lo + nsl + 1).
BATCHES = [(0, 6, 4), (4, 5, 3)]

SHEETW = 128 + 2 * KRAD   # sheet S[p, j] = ker(j - KRAD - p)


def _sheets_for_taps(taps):
    """taps: [KTAPS] zero-padded kernel. Returns [128, SHEETW] f32 sheet
    S[p, j] = tap(j - KRAD - p)."""
    assert taps.shape == (KTAPS,), taps.shape
    p = np.arange(128)[:, None]
    j = np.arange(SHEETW)[None, :]
    d = j - KRAD - p
    out = np.where(np.abs(d) <= KRAD,
                   np.take(np.pad(taps, 1), np.clip(d + KRAD, -1, KTAPS) + 1), 0.0)
    return out.astype(np.float32)


def _build_program():
    nc = bass.Bass("TRN2", target_bir_lowering=False)

    img = nc.dram_tensor("img", [H, W], F32, kind="ExternalInput")
    s1 = nc.dram_tensor("s1", [128, NG * SHEETW], F32, kind="ExternalInput")
    s2p = nc.dram_tensor("s2p", [128, ND * SHEETW], F32, kind="ExternalInput")
    s2m = nc.dram_tensor("s2m", [128, ND * SHEETW], F32, kind="ExternalInput")
    biasv = nc.dram_tensor("biasv", [128, NG], F32, kind="ExternalInput")
    minf = nc.dram_tensor("minf", [1, 1600], F32, kind="ExternalInput")
    zeros = nc.dram_tensor("zeros", [128, 512], F32, kind="ExternalInput")
    dog_out = nc.dram_tensor("dog_out", [NOUT, H, W], F32, kind="ExternalOutput")
    mask_out = nc.dram_tensor("mask_out", [NOUT, H, W], U8, kind="ExternalOutput")

    with tile.TileContext(nc) as tc:
        with (
            tc.tile_pool(name="gsheet", bufs=1) as gpool,
            tc.tile_pool(name="xblk", bufs=4) as xpool,
            tc.tile_pool(name="apool", bufs=6) as apool,
            tc.tile_pool(name="pp1", bufs=4, space="PSUM") as pp1,
            tc.tile_pool(name="pp2", bufs=2, space="PSUM") as pp2,
            tc.tile_pool(name="dog", bufs=2) as dpool,
            tc.tile_pool(name="sh", bufs=4) as shpool,
            tc.tile_pool(name="vchain", bufs=5) as vpool,
            tc.tile_pool(name="mask", bufs=2) as mpool,
            tc.tile_pool(name="misc", bufs=1) as miscpool,
        ):
            # ---- resident constants ----
            s1t = gpool.tile([128, NG * SHEETW], F32, tag="s1")
            nc.sync.dma_start(s1t[:], s1[:])
            s2pt = gpool.tile([128, ND * SHEETW], F32, tag="s2p")
            nc.sync.dma_start(s2pt[:], s2p[:])
            s2mt = gpool.tile([128, ND * SHEETW], F32, tag="s2m")
            nc.sync.dma_start(s2mt[:], s2m[:])
            biast = miscpool.tile([128, NG], F32, tag="bias")
            nc.sync.dma_start(biast[:], biasv[:])
            minft = miscpool.tile([1, 1600], F32, tag="minf")
            nc.sync.dma_start(minft[:], minf[:])
            zrt = miscpool.tile([128, 512], F32, tag="zeros")
            nc.sync.dma_start(zrt[:], zeros[:])

            def sheet1(g, a, b):
                return s1t[:, g * SHEETW + a: g * SHEETW + b]

            def sheet2(which, d, a, b):
                t = s2pt if which == "p" else s2mt
                return t[:, d * SHEETW + a: d * SHEETW + b]

            # pass2 kb-slice table: (kb, wh) -> out cols [c0, c1), sheet offset
            def p2_slices(wh):
                out = []
                for kb in range(8):
                    base = kb * 128 - wh * 512
                    c0 = max(0, base - KRAD)
                    c1 = min(512, base + 128 + KRAD)
                    if c1 <= c0:
                        continue
                    off = KRAD - base  # sheet col for out col c is off + c
                    out.append((kb, c0, c1, off + c0))
                return out

            P2SL = {0: p2_slices(0), 1: p2_slices(1)}

            for dog_lo, nsl, nmask in BATCHES:
                g_lo = dog_lo          # gaussian slots [g_lo, g_lo + nsl + 1)
                ngb = nsl + 1
                # stream X blocks for this batch
                xt = {}

                def xload(kb):
                    t = xpool.tile([128, W], F32, tag="x")
                    nc.sync.dma_start(t[:], img[kb * 128:(kb + 1) * 128, :])
                    return t

                xt[0] = xload(0)
                xt[1] = xload(1)

                a_t = {}        # local gaussian index -> A tile for current strip
                dog_t = {}      # strip -> dog tile [128, nsl, 1024]
                sh_dn = {}      # strip -> {wq: tile}
                WQ = 4

                def wq_rng(wq):
                    lo = max(0, wq * 256 - 1)
                    hi = min(W, wq * 256 + 257)
                    return lo, hi

                def pass1(gl, v):
                    """A^T for gaussian local slot gl, strip v -> A tile."""
                    g = g_lo + gl
                    at = apool.tile([128, 1024], F32, tag="a")
                    for m in range(8):
                        ps = pp1.tile([128, 128], F32, tag="ps1")
                        # center block kb == v; corners follow, last gets stop
                        nc.tensor.matmul(ps[:], xt[v][:, m * 128:(m + 1) * 128],
                                         sheet1(g, KRAD, KRAD + 128), start=True,
                                         stop=False)
                        if v > 0:
                            nc.tensor.matmul(ps[:, 0:KRAD],
                                             xt[v - 1][:, m * 128:(m + 1) * 128],
                                             sheet1(g, SHEETW - KRAD, SHEETW),
                                             start=False, stop=(v == 7))
                        if v < 7:
                            nc.tensor.matmul(ps[:, 128 - KRAD:128],
                                             xt[v + 1][:, m * 128:(m + 1) * 128],
                                             sheet1(g, 0, KRAD),
                                             start=False, stop=True)
                        assert v > 0 or v < 7
                        # evacuate PSUM -> A (halo slots add +-1e30 bias)
                        if g in (0, NG - 1):
                            nc.scalar.activation(at[:, m * 128:(m + 1) * 128], ps[:],
                                                 mybir.ActivationFunctionType.Identity,
                                                 bias=biast[:, g:g + 1], scale=1.0)
                        else:
                            nc.scalar.copy(at[:, m * 128:(m + 1) * 128], ps[:])
                    return at

                def pass2(dl, v, dtile):
                    """dog for local dog slot dl, strip v -> writes dtile[:, dl, :]."""
                    d = dog_lo + dl
                    for wh in range(2):
                        ps = pp2.tile([128, 512], F32, tag="ps2")
                        # full-width zero matmul opens the accumulation group
                        # (CoreSim requires uniform pending-zero per write)
                        nc.tensor.matmul(ps[:], xt[v][:, 0:128], zrt[:],
                                         start=True, stop=False)
                        first = False
                        for which, gl in (("p", dl), ("m", dl + 1)):
                            at = a_t[gl]
                            sl = P2SL[wh]
                            for i, (kb, c0, c1, so) in enumerate(sl):
                                lastg = which == "m" and i == len(sl) - 1
                                nc.tensor.matmul(
                                    ps[:, c0:c1],
                                    at[:, kb * 128:(kb + 1) * 128],
                                    sheet2(which, d, so, so + (c1 - c0)),
                                    start=first, stop=lastg)
                                first = False
                        nc.scalar.copy(dtile[:, dl, wh * 512:(wh + 1) * 512], ps[:])

                def build_sh_dn(v):
                    """SH_dn[v]: rows h-1 for strip v (needs dog_t[v], dog_t[v-1])."""
                    res = {}
                    for wq in range(WQ):
                        lo, hi = wq_rng(wq)
                        wd = hi - lo
                        t = shpool.tile([128, nsl, wd], F32, tag="shd")
                        nc.sync.dma_start(t[1:128, :, :], dog_t[v][0:127, :, lo:hi])
                        if v > 0:
                            nc.sync.dma_start(t[0:1, :, :],
                                              dog_t[v - 1][127:128, :, lo:hi])
                        else:
                            nc.sync.dma_start(t[0:1, :, :], minft[0:1, 0:nsl * wd])
                        res[wq] = t
                    return res

                def vector_stage(v):
                    dt = dog_t[v]
                    for wq in range(WQ):
                        lo, hi = wq_rng(wq)
                        wd = hi - lo
                        # SH_up
                        shu = shpool.tile([128, nsl, wd], F32, tag="shu")
                        nc.sync.dma_start(shu[0:127, :, :], dt[1:128, :, lo:hi])
                        if v < 7:
                            nc.sync.dma_start(shu[127:128, :, :],
                                              dog_t[v + 1][0:1, :, lo:hi])
                        else:
                            nc.sync.dma_start(shu[127:128, :, :],
                                              minft[0:1, 0:nsl * wd])
                        shd = sh_dn[v][wq]
                        # H-max
                        pt = vpool.tile([128, nsl, wd], F32, tag="vc")
                        nc.vector.tensor_tensor(pt[:], dt[:, :, lo:hi], shu[:],
                                                op=AX.max)
                        rt = vpool.tile([128, nsl, wd], F32, tag="vc")
                        nc.vector.tensor_tensor(rt[:], pt[:], shd[:], op=AX.max)
                        # scale-max
                        tt = vpool.tile([128, nsl - 1, wd], F32, tag="vc")
                        nc.vector.tensor_tensor(tt[:], rt[:, 0:nsl - 1, :],
                                                rt[:, 1:nsl, :], op=AX.max)
                        qt = vpool.tile([128, nmask, wd], F32, tag="vc")
                        nc.vector.tensor_tensor(qt[:], tt[:, 0:nmask, :],
                                                rt[:, 2:2 + nmask, :], op=AX.max)
                        # W-max
                        ut = vpool.tile([128, nmask, wd - 1], F32, tag="vc")
                        nc.vector.tensor_tensor(ut[:], qt[:, :, 0:wd - 1],
                                                qt[:, :, 1:wd], op=AX.max)
                        # pooled over the output range [wq*256, wq*256+256)
                        po = vpool.tile([128, nmask, 256], F32, tag="vc")
                        ow0 = wq * 256 - lo  # tile col of first output col
                        a = 1 if wq == 0 else 0      # skip w=0 in main op
                        b = 255 if wq == 3 else 256  # skip w=1023 in main op
                        # pooled[w] = max(U[w-1], U[w]); U tile col of w is w-lo
                        nc.vector.tensor_tensor(
                            po[:, :, a:b],
                            ut[:, :, ow0 + a - 1:ow0 + b - 1],
                            ut[:, :, ow0 + a:ow0 + b], op=AX.max)
                        if wq == 0:
                            nc.vector.tensor_copy(po[:, :, 0:1], ut[:, :, 0:1])
                        if wq == 3:
                            nc.vector.tensor_copy(po[:, :, 255:256],
                                                  ut[:, :, wd - 2:wd - 1])
                        # mask = (dog == pooled) & (dog > thr)
                        dslice = dt[:, 1:1 + nmask, wq * 256:wq * 256 + 256]
                        eqt = vpool.tile([128, nmask, 256], F32, tag="vc")
                        nc.vector.tensor_tensor(eqt[:], dslice, po[:],
                                                op=AX.is_equal)
                        mt = mpool.tile([128, nmask, 256], U8, tag="m")
                        nc.vector.scalar_tensor_tensor(
                            mt[:], dslice, float(THRESHOLD), eqt[:],
                            op0=AX.is_gt, op1=AX.logical_and)
                        # mask out
                        for j in range(nmask):
                            ch = dog_lo + 1 + j - 1   # global out channel index
                            nc.sync.dma_start(
                                mask_out[ch, v * 128:(v + 1) * 128,
                                         wq * 256:wq * 256 + 256],
                                mt[:, j, :])
                    # dog out (full width)
                    for j in range(nmask):
                        ch = dog_lo + 1 + j - 1
                        nc.sync.dma_start(
                            dog_out[ch, v * 128:(v + 1) * 128, :],
                            dt[:, 1 + j, :])

                for v in range(NSTRIP):
                    if v + 2 < NSTRIP and (v + 2) not in xt:
                        xt[v + 2] = xload(v + 2)
                    dt = dpool.tile([128, nsl, 1024], F32, tag="dog")
                    dog_t[v] = dt
                    for gl in range(ngb):
                        a_t[gl] = pass1(gl, v)
                        if gl >= 1:
                            pass2(gl - 1, v, dt)
                    sh_dn[v] = build_sh_dn(v)
                    if v >= 1:
                        vector_stage(v - 1)
                        del dog_t[v - 1]
                        del sh_dn[v - 1]
                vector_stage(NSTRIP - 1)
                dog_t.clear()
                sh_dn.clear()
                a_t.clear()
                xt.clear()

    return nc


def _split_multiwaits(nc):
    """This walrus build allows only one sem-wait per instruction; Tile's
    epilogue piles several onto one Drain. Split extras onto carrier Drains."""
    for f in nc.m.functions:
        for bb in f.blocks:
            out = []
            for ins in bb.instructions:
                si = ins.sync_info
                if si is not None and len(si.on_wait) > 1:
                    waits = list(si.on_wait)
                    for i, w in enumerate(waits[:-1]):
                        out.append(mybir.InstDrain(
                            name=f"{ins.name}-wsplit{i}",
                            engine=ins.engine,
                            debug=ins.debug,
                            sync_info=mybir.SyncInfo(on_wait=[w], on_update=[]),
                        ))
                    si.on_wait = waits[-1:]
                out.append(ins)
            bb.instructions = out


_CACHE = {}


def _program():
    if "nc" not in _CACHE:
        _CACHE["nc"] = _build_program()
    return _CACHE["nc"]


def _core_inputs(img2d, k1, sig, start):
    """Build the per-core input dict. start = first output dog channel."""
    s1 = np.zeros((NG, 128, SHEETW), np.float32)
    s2p = np.zeros((ND, 128, SHEETW), np.float32)
    s2m = np.zeros((ND, 128, SHEETW), np.float32)
    bias = np.zeros((128, NG), np.float32)
    for g in range(NG):
        gg = start - 1 + g
        s1[g] = _sheets_for_taps(k1[np.clip(gg, 0, NSIG - 1)])
        if gg < 0:
            bias[:, g] = -BIGBIAS
        elif gg >= NSIG:
            bias[:, g] = BIGBIAS
    for d in range(ND):
        gd = start - 1 + d
        sg = float(sig[np.clip(gd, 0, SIGMA_BINS - 1)])
        s2p[d] = sg * _sheets_for_taps(k1[np.clip(gd, 0, NSIG - 1)])
        s2m[d] = -sg * _sheets_for_taps(k1[np.clip(gd + 1, 0, NSIG - 1)])
    return {
        "img": np.ascontiguousarray(img2d, np.float32),
        "s1": np.ascontiguousarray(s1.transpose(1, 0, 2).reshape(128, -1)),
        "s2p": np.ascontiguousarray(s2p.transpose(1, 0, 2).reshape(128, -1)),
        "s2m": np.ascontiguousarray(s2m.transpose(1, 0, 2).reshape(128, -1)),
        "biasv": bias,
        "minf": np.full((1, 1600), NEG, np.float32),
        "zeros": np.zeros((128, 512), np.float32),
    }


def kernel(image, kernels_1d, sigmas):
    image = np.asarray(image, np.float32)
    k1 = np.asarray(kernels_1d, np.float32)
    sig = np.asarray(sigmas, np.float32)
    img2d = image[0, 0]

    nc = _program()
    _split_multiwaits(nc)  # idempotent; required for this walrus build
    in_maps = [_core_inputs(img2d, k1, sig, st) for st in STARTS]
    res = run_bass_kernel_spmd(nc, in_maps, core_ids=list(range(8)))

    dog = np.zeros((NDOG, H, W), np.float32)
    mask = np.zeros((NDOG, H, W), np.uint8)
    for c, st in enumerate(STARTS):
        dog[st:st + NOUT] = res.results[c]["dog_out"]
        mask[st:st + NOUT] = res.results[c]["mask_out"]
    return dog, mask.astype(bool)


if __name__ == "__main__":
    rng = np.random.default_rng(0)
    pass
